# Initial kernel scaffold
#
"""Causal self-attention (B=4, S=2048, E=1024, D=128, single head) on 8 TRN2 cores.

Sharding: core c = 2*b + h handles batch b; the two cores of a pair split the
causal key range by k-tile parity (h=0 even 128-row k-tiles, h=1 odd). All 8
cores run the *same* instruction stream (uniform SPMD program); per-core
differences live in DRAM data:
  - xt_kv / xt_oth [1024, 1024]: x[b].T columns gathered by s-tile parity
    (own-parity half feeds K/V projection; Q projection uses both)
  - codes [12, 512] bf16: per-subtile 0/-1e30 masks for the oth-parity
    attention blocks (parity-dependent, so host data)
Attention runs over parity-pure 512-row query blocks (kv-tile blocks first --
they only need the kv half, so they overlap the remaining DMA stream; then
oth-tile blocks). Diagonal masking for kv blocks uses h-independent triangle
tiles built on-device with affine_select; masks are pre-loaded into PSUM via
identity/rank-1 matmuls so the scores matmul accumulates onto them
(start=False), keeping masking off the DVE/ACT critical chain.
Each core emits unnormalized PV partials (pvT [128 d, 2048 q]) and softmax
denominators (sums [1, 2048]); the host combines the pair:
  out[b] = ((pv0 + pv1) / (s0 + s1)).T  (+ per-core q-column de-permutation)

All matmuls run in float32r (fp32 stored, 11-bit-mantissa PE reads) at full
PE rate; PSUM accumulation is fp32. Measured: rel err 3.7e-4, ~26.6 us/iter
steady-state on HW, 51.7 us single-shot in the cost model.
"""

import os

os.environ.setdefault("MYCRO_LOCAL_CACHE", "1")

import ml_dtypes
import numpy as np

B, S, E, D = 4, 2048, 1024, 128
P = 128
NT = S // P          # 16 global k-tiles per batch
LT = NT // 2         # 8 local (per-core) k-tiles
NQB = 4              # 512-wide query blocks
QBW = 512
NEB = E // P         # 8 e-tiles
SCALE = 1.0 / float(np.sqrt(D))
NEG = -1.0e30

TRACE = False        # set by test.py for profiling runs
TRACE_KW = {}

_CACHE = {}


def _build_module(reps=1):
    from contextlib import ExitStack

    import concourse.bacc as bacc
    import concourse.mybir as mybir
    import concourse.tile as tile

    f32 = mybir.dt.float32
    f32r = mybir.dt.float32r
    bf16 = mybir.dt.bfloat16

    nc = bacc.Bacc("TRN2", target_bir_lowering=False, debug=False, num_devices=8)

    xt_kv = nc.dram_tensor("xt_kv", [E, S // 2], f32r, kind="ExternalInput").ap()
    xt_oth = nc.dram_tensor("xt_oth", [E, S // 2], f32r, kind="ExternalInput").ap()
    wq_d = nc.dram_tensor("wq", [E, D], f32r, kind="ExternalInput").ap()
    wk_d = nc.dram_tensor("wk", [E, D], f32r, kind="ExternalInput").ap()
    wv_d = nc.dram_tensor("wv", [E, D], f32r, kind="ExternalInput").ap()
    bq_d = nc.dram_tensor("bq", [D], f32, kind="ExternalInput").ap()  # pre-scaled
    bk_d = nc.dram_tensor("bk", [D], f32, kind="ExternalInput").ap()
    bv_d = nc.dram_tensor("bv", [D], f32, kind="ExternalInput").ap()
    codes_d = nc.dram_tensor("codes", [12, QBW], bf16, kind="ExternalInput").ap()
    ident_d = nc.dram_tensor("ident", [P, P], f32r, kind="ExternalInput").ap()
    identb_d = nc.dram_tensor("identb", [P, P], bf16, kind="ExternalInput").ap()
    onesb_d = nc.dram_tensor("onesb", [1, P], bf16, kind="ExternalInput").ap()
    ones_d = nc.dram_tensor("ones", [P, 1], f32r, kind="ExternalInput").ap()
    pvt_d = nc.dram_tensor("pvt", [D, S], f32, kind="ExternalOutput").ap()
    sums_d = nc.dram_tensor("sums", [1, S], f32, kind="ExternalOutput").ap()

    with tile.TileContext(nc) as tc, ExitStack() as ctx:
        singles = ctx.enter_context(tc.tile_pool(name="singles", bufs=1))
        xpool = ctx.enter_context(tc.tile_pool(name="xpool", bufs=12))
        ppool = ctx.enter_context(tc.tile_pool(name="ppool", bufs=6))
        proj_ps = ctx.enter_context(tc.tile_pool(name="proj_ps", bufs=1, space="PSUM"))
        sc_ps = ctx.enter_context(tc.tile_pool(name="sc_ps", bufs=3, space="PSUM"))
        pv_ps = ctx.enter_context(tc.tile_pool(name="pv_ps", bufs=1, space="PSUM"))
        sum_ps = ctx.enter_context(tc.tile_pool(name="sum_ps", bufs=1, space="PSUM"))

        # ---- constants (ACT HWDGE ring; xt stream owns the SP ring) ----
        w_sb = {}
        for name, dram in (("wk", wk_d), ("wv", wv_d), ("wq", wq_d)):
            t = singles.tile([P, NEB, D], f32r, tag=f"w_{name}")
            nc.scalar.dma_start(t[:], dram.rearrange("(o p) d -> p o d", p=P))
            w_sb[name] = t
        b_sb = {}
        for name, dram in (("bq", bq_d), ("bk", bk_d), ("bv", bv_d)):
            t = singles.tile([P, 1], f32, tag=f"b_{name}")
            nc.scalar.dma_start(t[:], dram.rearrange("(p one) -> p one", one=1))
            b_sb[name] = t
        om = singles.tile([1, 12, QBW], bf16, tag="om")
        nc.scalar.dma_start(om[:], codes_d.rearrange("(one t) q -> one t q", one=1))
        # h-independent triangle masks for kv-pure blocks: tri[i] covers 4
        # q-subtiles; visible iff 256*(j - i) + qi - ki >= 0
        tri = singles.tile([P, 4, 4, P], bf16, tag="tri")
        nc.gpsimd.memset(tri[:], 0.0)
        for i in range(4):
            nc.gpsimd.affine_select(
                out=tri[:, i],
                in_=tri[:, i],
                pattern=[[256, 4], [1, P]],
                compare_op=mybir.AluOpType.is_ge,
                fill=NEG,
                base=-256 * i,
                channel_multiplier=-1,
            )
        ident = singles.tile([P, P], f32r, tag="ident")
        nc.scalar.dma_start(ident[:], ident_d[:])
        ones = singles.tile([P, 1], f32r, tag="ones")
        nc.scalar.dma_start(ones[:], ones_d[:])
        identb = singles.tile([P, P], bf16, tag="identb")
        nc.scalar.dma_start(identb[:], identb_d[:])
        onesb = singles.tile([1, P], bf16, tag="onesb")
        nc.scalar.dma_start(onesb[:], onesb_d[:])

        # ---- persistent activations ----
        kt = singles.tile([P, LT, P], f32r, tag="kt")      # K^T  [d, lt, k]
        vt = singles.tile([P, LT, P], f32r, tag="vt")      # V^T  [d, lt, s]
        vn = singles.tile([P, LT, D], f32r, tag="vn")      # V natural [s, lt, d]
        qt = singles.tile([P, 2, LT, P], f32r, tag="qt")   # Q^T [d, half, lt, q]
        pvt_sb = singles.tile([D, S], f32, tag="pvt_sb")
        sums_sb = singles.tile([1, S], f32, tag="sums_sb")

        def proj_kv_blk(sb):
            """K/V/Q projections for kv-half s-block sb (512 cols)."""
            xts = []
            for eo in range(NEB):
                xtile = xpool.tile([P, QBW], f32r, tag="xt")
                nc.sync.dma_start(
                    xtile[:], xt_kv[eo * P : (eo + 1) * P, sb * QBW : (sb + 1) * QBW]
                )
                xts.append(xtile)
            for name, dst, bias in (("wk", kt, "bk"), ("wv", vt, "bv")):
                ps = proj_ps.tile([P, QBW], f32, tag=f"ps_{name}")
                for eo in range(NEB):
                    nc.tensor.matmul(
                        ps[:],
                        w_sb[name][:, eo, :],
                        xts[eo][:],
                        start=(eo == 0),
                        stop=(eo == NEB - 1),
                    )
                dstv = dst.rearrange("p lt k -> p (lt k)")
                nc.vector.tensor_scalar_add(
                    dstv[:, sb * QBW : (sb + 1) * QBW], ps[:], b_sb[bias][:]
                )
            ps = proj_ps.tile([P, QBW], f32, tag="ps_q")
            for eo in range(NEB):
                nc.tensor.matmul(
                    ps[:],
                    w_sb["wq"][:, eo, :],
                    xts[eo][:],
                    start=(eo == 0),
                    stop=(eo == NEB - 1),
                )
            qv = qt.rearrange("p h lt k -> p (h lt k)")
            nc.vector.tensor_scalar(
                qv[:, sb * QBW : (sb + 1) * QBW],
                ps[:],
                SCALE,
                b_sb["bq"][:],
                mybir.AluOpType.mult,
                mybir.AluOpType.add,
            )

        def proj_q_oth(t0, ntiles):
            """Q projection for oth-local s-tiles [t0, t0+ntiles)."""
            w = ntiles * P
            xts = []
            for eo in range(NEB):
                xtile = xpool.tile([P, QBW], f32r, tag="xt")
                nc.sync.dma_start(
                    xtile[:, :w], xt_oth[eo * P : (eo + 1) * P, t0 * P : t0 * P + w]
                )
                xts.append(xtile)
            ps = proj_ps.tile([P, QBW], f32, tag="ps_q")
            for eo in range(NEB):
                nc.tensor.matmul(
                    ps[:, :w],
                    w_sb["wq"][:, eo, :],
                    xts[eo][:, :w],
                    start=(eo == 0),
                    stop=(eo == NEB - 1),
                )
            qv = qt.rearrange("p h lt k -> p (h lt k)")
            off = (S // 2) + t0 * P
            nc.vector.tensor_scalar(
                qv[:, off : off + w],
                ps[:, :w],
                SCALE,
                b_sb["bq"][:],
                mybir.AluOpType.mult,
                mybir.AluOpType.add,
            )

        def v_transpose(lt):
            ps = sc_ps.tile([P, P], f32r, tag="sc")
            nc.tensor.transpose(ps[:], vt[:, lt, :], ident[:])
            nc.vector.tensor_copy(out=vn[:, lt, :], in_=ps[:, :D])

        def attention_blk(half, blk):
            """Attention for q-block = {kv,oth}-local s-tiles [4*blk, 4*blk+4)."""
            nlt = 4 if blk == 0 else LT
            qview = qt[:, half, 4 * blk : 4 * blk + 4, :]     # [d, 4, 128] = 512 q
            col0 = (half * 2 + blk) * QBW
            pv = pv_ps.tile([P, QBW], f32, tag="pv")
            sm = sum_ps.tile([1, QBW], f32, tag="sm")
            for i in range(nlt):
                sc = sc_ps.tile([P, QBW], f32, tag="sc")
                if half == 0:
                    ti = i - (0 if blk == 0 else 4)
                    masked = ti >= 0
                    if masked:
                        nc.tensor.matmul(
                            sc[:], identb[:], tri[:, ti], start=True, stop=False
                        )
                else:
                    # ob2 k-tiles 0..3 are below every q-tile for both
                    # parities -- no mask needed there
                    masked = blk == 0 or i >= 4
                    if masked:
                        nc.tensor.matmul(
                            sc[:], onesb[:], om[0:1, 4 * blk + i, :],
                            start=True, stop=False,
                        )
                nc.tensor.matmul(
                    sc[:],
                    kt[:, i, :],
                    qview,
                    start=not masked,
                    stop=True,
                )
                p = ppool.tile([P, QBW], f32r, tag="p")
                nc.scalar.activation(p[:], sc[:], mybir.ActivationFunctionType.Exp)
                nc.tensor.matmul(
                    pv[:], vn[:, i, :], p[:], start=(i == 0), stop=(i == nlt - 1)
                )
                nc.tensor.matmul(
                    sm[:], ones[:], p[:], start=(i == 0), stop=(i == nlt - 1)
                )
            nc.vector.tensor_copy(out=pvt_sb[:, col0 : col0 + QBW], in_=pv[:])
            nc.vector.tensor_copy(out=sums_sb[:, col0 : col0 + QBW], in_=sm[:])
            out_eng = nc.gpsimd if (half * 2 + blk) < 3 else nc.sync
            out_eng.dma_start(
                pvt_d[:, col0 : col0 + QBW], pvt_sb[:, col0 : col0 + QBW]
            )
            out_eng.dma_start(
                sums_d[:, col0 : col0 + QBW], sums_sb[:, col0 : col0 + QBW]
            )

        # ---- emission order (priority hint for the scheduler) ----
        for _rep in range(reps):
            proj_kv_blk(0)
            for lt in range(4):
                v_transpose(lt)
            attention_blk(0, 0)
            proj_kv_blk(1)
            for lt in range(4, LT):
                v_transpose(lt)
            attention_blk(0, 1)
            proj_q_oth(0, 4)
            attention_blk(1, 0)
            proj_q_oth(4, 4)
            attention_blk(1, 1)

    nc.compile()
    return nc


def _get_module(reps=1):
    key = ("nc", reps)
    if key not in _CACHE:
        _CACHE[key] = _build_module(reps)
    return _CACHE[key]


def _host_prep(x, Wq, bq, Wk, bk, Wv, bv):
    """Build the 8 per-core input maps plus per-core q-column permutations."""
    x = np.asarray(x, dtype=np.float32)
    in_maps = []
    perms = []
    for c in range(8):
        b, h = divmod(c, 2)
        xt = np.ascontiguousarray(x[b].T)             # [E, S]
        xt3 = xt.reshape(E, NT, P)
        xt_kv = np.ascontiguousarray(xt3[:, h::2, :].reshape(E, S // 2))
        xt_oth = np.ascontiguousarray(xt3[:, 1 - h :: 2, :].reshape(E, S // 2))
        # oth-block codes: q-block = oth-local s-tiles; k-tile local i vs
        # q-subtile j: h=0 masked iff j < i_rel, h=1 masked iff j <= i_rel
        codes = np.zeros((12, NQB), dtype=np.float32)
        for idx in range(12):
            blk, i = (0, idx) if idx < 4 else (1, idx - 4)
            i_rel = i - (0 if blk == 0 else 4)
            for j in range(NQB):
                masked = (j < i_rel) if h == 0 else (j <= i_rel)
                if masked:
                    codes[idx, j] = NEG
        codes = np.repeat(codes, P, axis=1).astype(ml_dtypes.bfloat16)
        in_maps.append(
            {
                "xt_kv": xt_kv,
                "xt_oth": xt_oth,
                "wq": np.asarray(Wq, np.float32),
                "wk": np.asarray(Wk, np.float32),
                "wv": np.asarray(Wv, np.float32),
                "bq": np.asarray(bq, np.float32) * np.float32(SCALE),
                "bk": np.asarray(bk, np.float32),
                "bv": np.asarray(bv, np.float32),
                "codes": np.ascontiguousarray(codes),
                "ident": np.eye(P, dtype=np.float32),
                "identb": np.eye(P, dtype=ml_dtypes.bfloat16),
                "onesb": np.ones((1, P), dtype=ml_dtypes.bfloat16),
                "ones": np.ones((P, 1), dtype=np.float32),
            }
        )
        # storage col -> global q row: cols [0,1024) = kv-local tiles 0..7
        # (global tile 2j+h), cols [1024,2048) = oth tiles (global 2j+1-h)
        perm = np.empty(S, dtype=np.int64)
        for j in range(LT):
            perm[j * P : (j + 1) * P] = (2 * j + h) * P + np.arange(P)
            perm[(LT + j) * P : (LT + j + 1) * P] = (2 * j + 1 - h) * P + np.arange(P)
        perms.append(perm)
    return in_maps, perms


def kernel(x, Wq, bq, Wk, bk, Wv, bv):
    from concourse.bass_utils import run_bass_kernel_spmd

    nc = _get_module()
    in_maps, perms = _host_prep(x, Wq, bq, Wk, bk, Wv, bv)
    res = run_bass_kernel_spmd(
        nc,
        in_maps,
        core_ids=list(range(8)),
        trace=TRACE,
        **TRACE_KW,
    )
    _CACHE["last_result"] = res

    out = np.empty((B, S, D), dtype=np.float32)
    for b in range(B):
        r0, r1 = res.results[2 * b], res.results[2 * b + 1]
        pv = np.zeros((D, S), dtype=np.float64)
        sm = np.zeros((S,), dtype=np.float64)
        for r, perm in ((r0, perms[2 * b]), (r1, perms[2 * b + 1])):
            pv[:, perm] += r["pvt"].astype(np.float64)
            sm[perm] += r["sums"][0].astype(np.float64)
        out[b] = (pv / sm[None, :]).T.astype(np.float32)
    return out



# revision 1
# speedup vs baseline: 1.1016x; 1.1016x over previous
"""Causal self-attention (B=4, S=2048, E=1024, D=128, single head) on 8 TRN2 cores.

Sharding: core c = 2*b + h handles batch b; the two cores of a pair split the
causal key range by k-tile parity (h=0 even 128-row k-tiles, h=1 odd). All 8
cores run the *same* instruction stream (uniform SPMD program); per-core
differences live in DRAM data:
  - xt_kv / xt_oth [1024, 1024]: x[b].T columns gathered by s-tile parity
    (own-parity half feeds K/V projection; Q projection uses both)
  - codes [12, 512] bf16: per-subtile 0/-1e30 masks for the oth-parity
    attention blocks (parity-dependent, so host data)
Attention runs over parity-pure 512-row query blocks (kv-tile blocks first --
they only need the kv half, so they overlap the remaining DMA stream; then
oth-tile blocks). Diagonal masking for kv blocks uses h-independent triangle
tiles built on-device with affine_select; masks are pre-loaded into PSUM via
identity/rank-1 matmuls so the scores matmul accumulates onto them
(start=False), keeping masking off the DVE/ACT critical chain.
Each core emits unnormalized PV partials (pvT [128 d, 2048 q]) and softmax
denominators (sums [1, 2048]); the host combines the pair:
  out[b] = ((pv0 + pv1) / (s0 + s1)).T  (+ per-core q-column de-permutation)

All matmuls run in float32r (fp32 stored, 11-bit-mantissa PE reads) at full
PE rate; PSUM accumulation is fp32. Measured: rel err 3.7e-4, ~26.6 us/iter
steady-state on HW, 51.7 us single-shot in the cost model.
"""

import os

os.environ.setdefault("MYCRO_LOCAL_CACHE", "1")

import ml_dtypes
import numpy as np

B, S, E, D = 4, 2048, 1024, 128
P = 128
NT = S // P          # 16 global k-tiles per batch
LT = NT // 2         # 8 local (per-core) k-tiles
NQB = 4              # 512-wide query blocks
QBW = 512
NEB = E // P         # 8 e-tiles
SCALE = 1.0 / float(np.sqrt(D))
NEG = -1.0e30

TRACE = False        # set by test.py for profiling runs
TRACE_KW = {}

_CACHE = {}


def _build_module(reps=1):
    from contextlib import ExitStack

    import concourse.bacc as bacc
    import concourse.mybir as mybir
    import concourse.tile as tile

    f32 = mybir.dt.float32
    f32r = mybir.dt.float32r
    bf16 = mybir.dt.bfloat16

    nc = bacc.Bacc("TRN2", target_bir_lowering=False, debug=False, num_devices=8)

    xt_kv = nc.dram_tensor("xt_kv", [E, S // 2], f32r, kind="ExternalInput").ap()
    xt_oth = nc.dram_tensor("xt_oth", [E, S // 2], f32r, kind="ExternalInput").ap()
    wq_d = nc.dram_tensor("wq", [E, D], f32r, kind="ExternalInput").ap()
    wk_d = nc.dram_tensor("wk", [E, D], f32r, kind="ExternalInput").ap()
    wv_d = nc.dram_tensor("wv", [E, D], f32r, kind="ExternalInput").ap()
    bq_d = nc.dram_tensor("bq", [D], f32, kind="ExternalInput").ap()  # pre-scaled
    bk_d = nc.dram_tensor("bk", [D], f32, kind="ExternalInput").ap()
    bv_d = nc.dram_tensor("bv", [D], f32, kind="ExternalInput").ap()
    codes_d = nc.dram_tensor("codes", [12, QBW], bf16, kind="ExternalInput").ap()
    ident_d = nc.dram_tensor("ident", [P, P], f32r, kind="ExternalInput").ap()
    identb_d = nc.dram_tensor("identb", [P, P], bf16, kind="ExternalInput").ap()
    onesb_d = nc.dram_tensor("onesb", [1, P], bf16, kind="ExternalInput").ap()
    ones_d = nc.dram_tensor("ones", [P, 1], f32r, kind="ExternalInput").ap()
    pvt_d = nc.dram_tensor("pvt", [D, S], f32, kind="ExternalOutput").ap()
    sums_d = nc.dram_tensor("sums", [1, S], f32, kind="ExternalOutput").ap()

    with tile.TileContext(nc) as tc, ExitStack() as ctx:
        singles = ctx.enter_context(tc.tile_pool(name="singles", bufs=1))
        xpool = ctx.enter_context(tc.tile_pool(name="xpool", bufs=12))
        ppool = ctx.enter_context(tc.tile_pool(name="ppool", bufs=6))
        proj_ps = ctx.enter_context(tc.tile_pool(name="proj_ps", bufs=1, space="PSUM"))
        sc_ps = ctx.enter_context(tc.tile_pool(name="sc_ps", bufs=3, space="PSUM"))
        pv_ps = ctx.enter_context(tc.tile_pool(name="pv_ps", bufs=1, space="PSUM"))
        sum_ps = ctx.enter_context(tc.tile_pool(name="sum_ps", bufs=1, space="PSUM"))

        # ---- constants (ACT HWDGE ring; xt stream owns the SP ring) ----
        w_sb = {}
        for name, dram in (("wk", wk_d), ("wv", wv_d), ("wq", wq_d)):
            t = singles.tile([P, NEB, D], f32r, tag=f"w_{name}")
            nc.scalar.dma_start(t[:], dram.rearrange("(o p) d -> p o d", p=P))
            w_sb[name] = t
        b_sb = {}
        for name, dram in (("bq", bq_d), ("bk", bk_d), ("bv", bv_d)):
            t = singles.tile([P, 1], f32, tag=f"b_{name}")
            nc.scalar.dma_start(t[:], dram.rearrange("(p one) -> p one", one=1))
            b_sb[name] = t
        om = singles.tile([1, 12, QBW], bf16, tag="om")
        nc.scalar.dma_start(om[:], codes_d.rearrange("(one t) q -> one t q", one=1))
        # h-independent triangle masks for kv-pure blocks: tri[i] covers 4
        # q-subtiles; visible iff 256*(j - i) + qi - ki >= 0
        tri = singles.tile([P, 4, 4, P], bf16, tag="tri")
        nc.gpsimd.memset(tri[:], 0.0)
        for i in range(4):
            nc.gpsimd.affine_select(
                out=tri[:, i],
                in_=tri[:, i],
                pattern=[[256, 4], [1, P]],
                compare_op=mybir.AluOpType.is_ge,
                fill=NEG,
                base=-256 * i,
                channel_multiplier=-1,
            )
        ident = singles.tile([P, P], f32r, tag="ident")
        nc.scalar.dma_start(ident[:], ident_d[:])
        ones = singles.tile([P, 1], f32r, tag="ones")
        nc.scalar.dma_start(ones[:], ones_d[:])
        identb = singles.tile([P, P], bf16, tag="identb")
        nc.scalar.dma_start(identb[:], identb_d[:])
        onesb = singles.tile([1, P], bf16, tag="onesb")
        nc.scalar.dma_start(onesb[:], onesb_d[:])

        # ---- persistent activations ----
        kt = singles.tile([P, LT, P], f32r, tag="kt")      # K^T  [d, lt, k]
        vt = singles.tile([P, LT, P], f32r, tag="vt")      # V^T  [d, lt, s]
        vn = singles.tile([P, LT, D], f32r, tag="vn")      # V natural [s, lt, d]
        qt = singles.tile([P, 2, LT, P], f32r, tag="qt")   # Q^T [d, half, lt, q]
        pvt_sb = singles.tile([D, S], f32, tag="pvt_sb")
        sums_sb = singles.tile([1, S], f32, tag="sums_sb")

        def proj_kv_blk(sb):
            """K/V/Q projections for kv-half s-block sb (512 cols)."""
            xts = []
            for eo in range(NEB):
                xtile = xpool.tile([P, QBW], f32r, tag="xt")
                nc.sync.dma_start(
                    xtile[:], xt_kv[eo * P : (eo + 1) * P, sb * QBW : (sb + 1) * QBW]
                )
                xts.append(xtile)
            for name, dst, bias in (("wk", kt, "bk"), ("wv", vt, "bv")):
                ps = proj_ps.tile([P, QBW], f32, tag=f"ps_{name}")
                for eo in range(NEB):
                    nc.tensor.matmul(
                        ps[:],
                        w_sb[name][:, eo, :],
                        xts[eo][:],
                        start=(eo == 0),
                        stop=(eo == NEB - 1),
                    )
                dstv = dst.rearrange("p lt k -> p (lt k)")
                nc.vector.tensor_scalar_add(
                    dstv[:, sb * QBW : (sb + 1) * QBW], ps[:], b_sb[bias][:]
                )
            ps = proj_ps.tile([P, QBW], f32, tag="ps_q")
            for eo in range(NEB):
                nc.tensor.matmul(
                    ps[:],
                    w_sb["wq"][:, eo, :],
                    xts[eo][:],
                    start=(eo == 0),
                    stop=(eo == NEB - 1),
                )
            qv = qt.rearrange("p h lt k -> p (h lt k)")
            nc.vector.tensor_scalar(
                qv[:, sb * QBW : (sb + 1) * QBW],
                ps[:],
                SCALE,
                b_sb["bq"][:],
                mybir.AluOpType.mult,
                mybir.AluOpType.add,
            )

        def proj_q_oth(t0, ntiles):
            """Q projection for oth-local s-tiles [t0, t0+ntiles)."""
            w = ntiles * P
            xts = []
            for eo in range(NEB):
                xtile = xpool.tile([P, QBW], f32r, tag="xt")
                nc.sync.dma_start(
                    xtile[:, :w], xt_oth[eo * P : (eo + 1) * P, t0 * P : t0 * P + w]
                )
                xts.append(xtile)
            ps = proj_ps.tile([P, QBW], f32, tag="ps_q")
            for eo in range(NEB):
                nc.tensor.matmul(
                    ps[:, :w],
                    w_sb["wq"][:, eo, :],
                    xts[eo][:, :w],
                    start=(eo == 0),
                    stop=(eo == NEB - 1),
                )
            qv = qt.rearrange("p h lt k -> p (h lt k)")
            off = (S // 2) + t0 * P
            nc.vector.tensor_scalar(
                qv[:, off : off + w],
                ps[:, :w],
                SCALE,
                b_sb["bq"][:],
                mybir.AluOpType.mult,
                mybir.AluOpType.add,
            )

        def v_transpose(lt):
            ps = sc_ps.tile([P, P], f32r, tag="sc")
            nc.tensor.transpose(ps[:], vt[:, lt, :], ident[:])
            nc.vector.tensor_copy(out=vn[:, lt, :], in_=ps[:, :D])

        def attention_blk(half, blk):
            """Attention for q-block = {kv,oth}-local s-tiles [4*blk, 4*blk+4)."""
            nlt = 4 if blk == 0 else LT
            qview = qt[:, half, 4 * blk : 4 * blk + 4, :]     # [d, 4, 128] = 512 q
            col0 = (half * 2 + blk) * QBW
            pv = pv_ps.tile([P, QBW], f32, tag="pv")
            sm = sum_ps.tile([1, QBW], f32, tag="sm")
            for i in range(nlt):
                sc = sc_ps.tile([P, QBW], f32, tag="sc")
                if half == 0:
                    ti = i - (0 if blk == 0 else 4)
                    masked = ti >= 0
                    if masked:
                        nc.tensor.matmul(
                            sc[:], identb[:], tri[:, ti], start=True, stop=False
                        )
                else:
                    # ob2 k-tiles 0..3 are below every q-tile for both
                    # parities -- no mask needed there
                    masked = blk == 0 or i >= 4
                    if masked:
                        nc.tensor.matmul(
                            sc[:], onesb[:], om[0:1, 4 * blk + i, :],
                            start=True, stop=False,
                        )
                nc.tensor.matmul(
                    sc[:],
                    kt[:, i, :],
                    qview,
                    start=not masked,
                    stop=True,
                )
                p = ppool.tile([P, QBW], f32r, tag="p")
                nc.scalar.activation(p[:], sc[:], mybir.ActivationFunctionType.Exp)
                nc.tensor.matmul(
                    pv[:], vn[:, i, :], p[:], start=(i == 0), stop=(i == nlt - 1)
                )
                nc.tensor.matmul(
                    sm[:], ones[:], p[:], start=(i == 0), stop=(i == nlt - 1)
                )
            nc.vector.tensor_copy(out=pvt_sb[:, col0 : col0 + QBW], in_=pv[:])
            nc.vector.tensor_copy(out=sums_sb[:, col0 : col0 + QBW], in_=sm[:])
            out_eng = nc.gpsimd if (half * 2 + blk) < 3 else nc.sync
            out_eng.dma_start(
                pvt_d[:, col0 : col0 + QBW], pvt_sb[:, col0 : col0 + QBW]
            )
            out_eng.dma_start(
                sums_d[:, col0 : col0 + QBW], sums_sb[:, col0 : col0 + QBW]
            )

        # ---- emission order (priority hint for the scheduler) ----
        for _rep in range(reps):
            proj_kv_blk(0)
            for lt in range(4):
                v_transpose(lt)
            attention_blk(0, 0)
            proj_kv_blk(1)
            for lt in range(4, LT):
                v_transpose(lt)
            attention_blk(0, 1)
            proj_q_oth(0, 4)
            attention_blk(1, 0)
            proj_q_oth(4, 4)
            attention_blk(1, 1)

    nc.compile()
    return nc


def _get_module(reps=1):
    key = ("nc", reps)
    if key not in _CACHE:
        _CACHE[key] = _build_module(reps)
    return _CACHE[key]


def _host_prep(x, Wq, bq, Wk, bk, Wv, bv):
    """Build the 8 per-core input maps plus per-core q-column permutations."""
    x = np.asarray(x, dtype=np.float32)
    in_maps = []
    perms = []
    for c in range(8):
        b, h = divmod(c, 2)
        xt = np.ascontiguousarray(x[b].T)             # [E, S]
        xt3 = xt.reshape(E, NT, P)
        xt_kv = np.ascontiguousarray(xt3[:, h::2, :].reshape(E, S // 2))
        xt_oth = np.ascontiguousarray(xt3[:, 1 - h :: 2, :].reshape(E, S // 2))
        # oth-block codes: q-block = oth-local s-tiles; k-tile local i vs
        # q-subtile j: h=0 masked iff j < i_rel, h=1 masked iff j <= i_rel
        codes = np.zeros((12, NQB), dtype=np.float32)
        for idx in range(12):
            blk, i = (0, idx) if idx < 4 else (1, idx - 4)
            i_rel = i - (0 if blk == 0 else 4)
            for j in range(NQB):
                masked = (j < i_rel) if h == 0 else (j <= i_rel)
                if masked:
                    codes[idx, j] = NEG
        codes = np.repeat(codes, P, axis=1).astype(ml_dtypes.bfloat16)
        in_maps.append(
            {
                "xt_kv": xt_kv,
                "xt_oth": xt_oth,
                "wq": np.asarray(Wq, np.float32),
                "wk": np.asarray(Wk, np.float32),
                "wv": np.asarray(Wv, np.float32),
                "bq": np.asarray(bq, np.float32) * np.float32(SCALE),
                "bk": np.asarray(bk, np.float32),
                "bv": np.asarray(bv, np.float32),
                "codes": np.ascontiguousarray(codes),
                "ident": np.eye(P, dtype=np.float32),
                "identb": np.eye(P, dtype=ml_dtypes.bfloat16),
                "onesb": np.ones((1, P), dtype=ml_dtypes.bfloat16),
                "ones": np.ones((P, 1), dtype=np.float32),
            }
        )
        # storage col -> global q row: cols [0,1024) = kv-local tiles 0..7
        # (global tile 2j+h), cols [1024,2048) = oth tiles (global 2j+1-h)
        perm = np.empty(S, dtype=np.int64)
        for j in range(LT):
            perm[j * P : (j + 1) * P] = (2 * j + h) * P + np.arange(P)
            perm[(LT + j) * P : (LT + j + 1) * P] = (2 * j + 1 - h) * P + np.arange(P)
        perms.append(perm)
    return in_maps, perms


def kernel(x, Wq, bq, Wk, bk, Wv, bv):
    from concourse.bass_utils import run_bass_kernel_spmd

    nc = _get_module()
    in_maps, perms = _host_prep(x, Wq, bq, Wk, bk, Wv, bv)
    res = run_bass_kernel_spmd(
        nc,
        in_maps,
        core_ids=list(range(8)),
        trace=TRACE,
        **TRACE_KW,
    )
    _CACHE["last_result"] = res

    out = np.empty((B, S, D), dtype=np.float32)
    for b in range(B):
        r0, r1 = res.results[2 * b], res.results[2 * b + 1]
        pv = np.zeros((D, S), dtype=np.float64)
        sm = np.zeros((S,), dtype=np.float64)
        for r, perm in ((r0, perms[2 * b]), (r1, perms[2 * b + 1])):
            pv[:, perm] += r["pvt"].astype(np.float64)
            sm[perm] += r["sums"][0].astype(np.float64)
        out[b] = (pv / sm[None, :]).T.astype(np.float32)
    return out

